# revision 28
# baseline (speedup 1.0000x reference)
"""Trainium2 Bass kernel for nn_Attention (B=8, L=2048, D=512).

Strategy: data-parallel over batch — one batch element per NeuronCore
(8 cores). The host feeds each core its batch slice transposed and
pre-cast to bf16 (the on-device matmuls run bf16 anyway, so this is
pure data marshaling that halves the DMA bytes), plus weight-only
precomputations (all activation arithmetic runs on device):
  - softmax is shift-invariant, so q.k = (x wq^T + bq).(s wk^T + bk)
    reduces to x A s^T + sw[k] with A = wq^T wk and sw = s.(bq wk)/sqrt(D)
    — the query-constant terms drop. This removes one of the two score
    projections entirely.
Per core:
  - a short burst of warmup matmuls keeps the PE HAM activity window
    busy while the first DMAs land, so real matmuls start at 2.4 GHz
    instead of the cold 1.2 GHz half-clock
  - inputs stream straight into persistent bf16 SBUF tiles (no
    on-device casts): states + input on the sync HWDGE queue, weights
    on the scalar HWDGE queue
  - T^T = A^T x^T (transposed layout); V = s wv^T (natural layout, no
    bias: softmax rows sum to 1, so the v-bias is equivalent to adding
    bv to the context at the end)
  - sw = s.(bq wk)/sqrt(D) is computed on DVE (per-chunk fused
    multiply-adds) + four ones-stationary reduce matmuls, then scattered
    into per-partition layout via a tiny DRAM round-trip on the scalar
    HWDGE queue, well before the first exp needs it
  - scores^T = s^T-stationary x T^T-moving  => [k, q] layout, so the
    softmax key-dim lands on partitions
  - E = exp(scale * scores^T + sw[k]) on ScalarE, sw as the
    per-partition activation bias (no max-subtraction needed:
    shift-invariance again, and scores are O(1) here)
  - key-dim sums: the 16 E^T tiles are accumulated on DVE (bf16
    partials; their rounding averages out over the 128 partitions the
    matmul then sums), one ones-stationary matmul -> [1, q] row,
    re-oriented per q-block with a tiny DRAM round-trip scatter (no PE
    transposes)
  - context = (E^T.T @ V) * recip(sums) + bv via one fused DVE op
All matmuls run in bf16 with fp32 PSUM accumulation.

The mask input is all-ones per the problem spec; kernel() verifies that
on the host and falls back to an exact numpy implementation for any
other mask. A per-batch spot-check guards the device path (retry, then
exact-host fallback) so out-of-spec inputs or a bad run can never
return wrong results.
"""

import numpy as np

B, L, D = 8, 2048, 512
P = 128
LT = L // P  # 16 l-tiles
DC = D // P  # 4 d/e chunks
NQ = 512  # q-block width
QB = L // NQ  # 4 q blocks
NB = L // NQ  # 4 l-blocks (512 rows each)
N_CORES = 8
N_WARM = 48  # HAM warmup matmuls (N=128 each, ~110ns cold)
SCALE = 1.0 / float(np.sqrt(D))

_cache = {}


def _build_fast():
    import concourse.tile as tile
    from concourse import bacc, mybir
    from concourse.bass import ds

    F32 = mybir.dt.float32
    BF16 = mybir.dt.bfloat16
    AF = mybir.ActivationFunctionType
    ALU = mybir.AluOpType

    nc = bacc.Bacc(
        "TRN2", target_bir_lowering=False, debug=False, num_devices=N_CORES
    )
    # all matrices host-packed to [partition, chunk, col] so every DMA
    # line is >=2KB contiguous and lands straight in the SBUF tiles
    xT_ext = nc.dram_tensor("inputT", [P, DC, L], BF16, kind="ExternalInput")
    sT_ext = nc.dram_tensor("statesT", [P, DC, L], BF16, kind="ExternalInput")
    # amat = wq.T @ wk (scores reduce to x @ amat @ s.T plus a per-key
    # bias; the query-constant terms drop out of the softmax)
    amat_ext = nc.dram_tensor("amat", [P, DC, D], BF16, kind="ExternalInput")
    wvT_ext = nc.dram_tensor("wvT", [P, DC, D], BF16, kind="ExternalInput")
    # wvec = (bq @ wk) * scale; bv fed separately
    wvec_ext = nc.dram_tensor("wvec", [D], F32, kind="ExternalInput")
    bv_ext = nc.dram_tensor("bv", [D], F32, kind="ExternalInput")
    out_ext = nc.dram_tensor("out", [L, D], BF16, kind="ExternalOutput")

    with tile.TileContext(nc) as tc:
        with (
            tc.tile_pool(name="consts", bufs=1) as consts,
            tc.tile_pool(name="persist", bufs=1) as persist,
            tc.tile_pool(name="et", bufs=2) as et_pool,
            tc.tile_pool(name="outp", bufs=3) as outp,
            tc.tile_pool(name="dscr", bufs=2, space="DRAM") as dscr,
            tc.tile_pool(name="psum_mm", bufs=4, space="PSUM") as psum_mm,
            tc.tile_pool(name="psum_u", bufs=3, space="PSUM") as psum_u,
            tc.tile_pool(name="psum_row", bufs=1, space="PSUM") as psum_row,
        ):
            # ---- PE warmup: trip the HAM activity window while DMAs
            # stream, so phase-A matmuls run at full clock ----
            warm_st = consts.tile([P, P], BF16, tag="warm_st")
            nc.gpsimd.memset(warm_st[:], 1.0)
            warm_ps = psum_row.tile([1, P], F32, tag="row", name="warm_ps")
            for _ in range(N_WARM):
                nc.tensor.matmul(
                    warm_ps[:],
                    warm_st[:, ds(0, 1)],
                    warm_st[:],
                    start=True,
                    stop=True,
                )
            # liveness anchor so the warmup chain can't be pruned
            warm_sb = consts.tile([1, 1], F32, tag="warm_sb")
            nc.vector.tensor_copy(warm_sb[:], warm_ps[:, ds(0, 1)])
            warm_dr = dscr.tile([1], F32, tag="warm_dr")
            nc.gpsimd.dma_start(
                warm_dr[:].rearrange("(one o) -> one o", one=1), warm_sb[:]
            )

            ones_st = consts.tile([1, P], BF16, tag="ones_st")
            nc.gpsimd.memset(ones_st[:], 1.0)
            ones_mv = consts.tile([P, 1], BF16, tag="ones_mv")
            nc.gpsimd.memset(ones_mv[:], 1.0)

            # persistent bf16 tensors (DMA lands directly in these)
            xT = persist.tile([P, DC, L], BF16, tag="xT")
            sT = persist.tile([P, DC, L], BF16, tag="sT")
            TT = persist.tile([P, DC, L], BF16, tag="TT")
            V = persist.tile([P, LT, D], BF16, tag="V")
            amat = persist.tile([P, DC, D], BF16, tag="amat")
            wvT = persist.tile([P, DC, D], BF16, tag="wvT")
            sw_sb = persist.tile([P, LT], F32, tag="sw_sb")

            # ---- input DMAs, interleaved across the two HWDGE rings
            # so phase A's states feed outruns PE consumption ----
            HL = L // 2

            # Both HWDGE rings share the 16 SDMA engines (~125 GB/s
            # each when both run), so states quarters alternate across
            # the rings strictly ahead of everything except wv.
            # sync ring: s q0, s q2, then input halves
            nc.sync.dma_start(
                sT[:, :, ds(0, NQ)], sT_ext.ap()[:, :, ds(0, NQ)]
            )
            nc.sync.dma_start(
                sT[:, :, ds(2 * NQ, NQ)], sT_ext.ap()[:, :, ds(2 * NQ, NQ)]
            )
            nc.sync.dma_start(
                xT[:, :, ds(0, HL)], xT_ext.ap()[:, :, ds(0, HL)]
            )
            nc.sync.dma_start(
                xT[:, :, ds(HL, HL)], xT_ext.ap()[:, :, ds(HL, HL)]
            )
            # scalar ring: wv (gates phase A), s q1, s q3, amat, vectors
            nc.scalar.dma_start(wvT[:], wvT_ext.ap())
            nc.scalar.dma_start(
                sT[:, :, ds(NQ, NQ)], sT_ext.ap()[:, :, ds(NQ, NQ)]
            )
            nc.scalar.dma_start(
                sT[:, :, ds(3 * NQ, NQ)], sT_ext.ap()[:, :, ds(3 * NQ, NQ)]
            )
            nc.scalar.dma_start(amat[:], amat_ext.ap())
            wvec_sb = consts.tile([P, DC], F32, tag="wvec")
            nc.scalar.dma_start(
                wvec_sb[:], wvec_ext.ap().rearrange("(c p) -> p c", p=P)
            )
            bv_f32 = consts.tile([1, D], F32, tag="bv_f32")
            nc.scalar.dma_start(
                bv_f32[:], bv_ext.ap().rearrange("(one d) -> one d", one=1)
            )
            bv_bf = consts.tile([1, D], BF16, tag="bv_bf")
            nc.vector.tensor_copy(bv_bf[:], bv_f32[:])

            # ---- Phase A: V projection ----
            for t in range(LT):
                ps = psum_mm.tile([P, D], F32, tag="ps_mm")
                for c in range(DC):
                    nc.tensor.matmul(
                        ps[:],
                        sT[:, c, ds(t * P, P)],
                        wvT[:, c, :],
                        start=(c == 0),
                        stop=(c == DC - 1),
                    )
                nc.vector.tensor_copy(V[:, t, :], ps[:])

            # ---- sw = s.(bq wk)*scale on DVE + 4 reduce matmuls ----
            # (the host-packed states halves land by ~11us, so the DVE
            # STT chain's wait resolves before it could head-of-line
            # block the V copies; bf16 partials are plenty: sw is a
            # tiny exp-bias ~O(0.03))
            swp = persist.tile([P, L], BF16, tag="swp")
            nc.vector.tensor_scalar_mul(
                swp[:], sT[:, 0, :], wvec_sb[:, ds(0, 1)]
            )
            for c in range(1, DC):
                nc.vector.scalar_tensor_tensor(
                    swp[:],
                    sT[:, c, :],
                    wvec_sb[:, ds(c, 1)],
                    swp[:],
                    op0=ALU.mult,
                    op1=ALU.add,
                )
            swrow = consts.tile([1, L], F32, tag="swrow")
            for b4 in range(NB):
                rps = psum_row.tile([1, NQ], F32, tag="row")
                nc.tensor.matmul(
                    rps[:], ones_mv[:, :], swp[:, ds(b4 * NQ, NQ)],
                    start=True, stop=True,
                )
                nc.vector.tensor_copy(swrow[:, ds(b4 * NQ, NQ)], rps[:])
            # scatter [1, L] -> [128, LT] via a tiny DRAM round-trip on
            # the sync HWDGE ring (only the phase-C output DMAs queue
            # behind it, and they aren't needed until much later)
            sw_dr = dscr.tile([L], F32, tag="sw_dr")
            nc.sync.dma_start(
                sw_dr[:].rearrange("(one w) -> one w", one=1), swrow[:]
            )
            nc.sync.dma_start(
                sw_sb[:], sw_dr[:].rearrange("(t p) -> p t", p=P)
            )

            # BV: bv broadcast to all 128 partitions (ones-column matmul)
            bv_ps = psum_u.tile([P, D], F32, tag="ps_u", name="bv_ps")
            nc.tensor.matmul(
                bv_ps[:], ones_st[:, :], bv_bf[:, :], start=True, stop=True
            )
            BV = consts.tile([P, D], F32, tag="BV")
            nc.vector.tensor_copy(BV[:], bv_ps[:])

            # ---- Phase B: T^T = amat.T-projection of x (no bias) ----
            for lb in range(NB):
                for e in range(DC):
                    ps = psum_mm.tile([P, NQ], F32, tag="ps_mm")
                    for c in range(DC):
                        nc.tensor.matmul(
                            ps[:],
                            amat[:, c, ds(e * P, P)],
                            xT[:, c, ds(lb * NQ, NQ)],
                            start=(c == 0),
                            stop=(c == DC - 1),
                        )
                    nc.scalar.copy(TT[:, e, ds(lb * NQ, NQ)], ps[:])

            # ---- Phase C: attention, per q-block ----
            for qb in range(QB):
                ET = et_pool.tile([P, LT, NQ], BF16, tag="ET")
                # key-dim sums accumulate on DVE as each exp lands (the
                # bf16 partials' rounding averages out across the 128
                # partitions summed by the matmul)
                acc = outp.tile([P, NQ], BF16, tag="tsum", bufs=2)
                for kt in range(LT):
                    ps = psum_mm.tile([P, NQ], F32, tag="ps_mm")
                    for e in range(DC):
                        nc.tensor.matmul(
                            ps[:],
                            sT[:, e, ds(kt * P, P)],
                            TT[:, e, ds(qb * NQ, NQ)],
                            start=(e == 0),
                            stop=(e == DC - 1),
                        )
                    nc.scalar.activation(
                        ET[:, kt, :],
                        ps[:],
                        AF.Exp,
                        bias=sw_sb[:, ds(kt, 1)],
                        scale=SCALE,
                    )
                    if kt == 1:
                        nc.vector.tensor_tensor(
                            acc[:], ET[:, 0, :], ET[:, 1, :],
                            ALU.add,
                        )
                    elif kt > 1:
                        nc.vector.tensor_tensor(
                            acc[:], acc[:], ET[:, kt, :],
                            ALU.add,
                        )

                row_ps = psum_row.tile([1, NQ], F32, tag="row")
                nc.tensor.matmul(
                    row_ps[:], ones_mv[:, :], acc[:], start=True, stop=True
                )
                row_sb = outp.tile([1, NQ], F32, tag="row_sb")
                nc.vector.tensor_copy(row_sb[:], row_ps[:])
                # reorient the sums [1, NQ] -> [128, NQ//P] via DRAM
                rrow = dscr.tile([NQ], F32, tag="rec_dr")
                nc.scalar.dma_start(
                    rrow[:].rearrange("(one w) -> one w", one=1), row_sb[:]
                )
                rec_in = outp.tile([P, NQ // P], F32, tag="rec_in", bufs=2)
                nc.scalar.dma_start(
                    rec_in[:], rrow[:].rearrange("(j p) -> p j", p=P)
                )
                rec = outp.tile([P, NQ // P], F32, tag="rec", bufs=2)
                nc.vector.reciprocal(rec[:], rec_in[:])

                for jp in range(NQ // P // 2):
                    # bf16 output pair: two q-tiles share one DMA
                    o = outp.tile([P, 2, D], BF16, tag="o")
                    for u in range(2):
                        j = jp * 2 + u
                        u_ps = psum_u.tile([P, D], F32, tag="ps_u")
                        for kt in range(LT):
                            nc.tensor.matmul(
                                u_ps[:],
                                ET[:, kt, ds(j * P, P)],
                                V[:, kt, :],
                                start=(kt == 0),
                                stop=(kt == LT - 1),
                            )
                        nc.vector.scalar_tensor_tensor(
                            o[:, u, :],
                            u_ps[:],
                            rec[:, ds(j, 1)],
                            BV[:],
                            op0=ALU.mult,
                            op1=ALU.add,
                        )
                    nc.sync.dma_start(
                        out_ext.ap()[
                            ds((qb * 2 + jp) * 2 * P, 2 * P), :
                        ].rearrange("(u p) e -> p u e", p=P),
                        o[:],
                    )

    nc.compile()
    return nc


def _pack_pcl(mT):
    """[D, cols] -> [P, DC, cols] with row d = c*P + p mapped to [p, c]."""
    cols = mT.shape[1]
    return np.ascontiguousarray(
        mT.reshape(DC, P, cols).transpose(1, 0, 2)
    )


def _make_in_maps(input, states, wq, bq, wk, bk, wv, bv):
    import ml_dtypes

    bf16 = ml_dtypes.bfloat16
    wq64 = np.asarray(wq, dtype=np.float64)
    wk64 = np.asarray(wk, dtype=np.float64)
    amat = _pack_pcl((wq64.T @ wk64).astype(bf16))
    wvec = np.ascontiguousarray(
        ((np.asarray(bq, dtype=np.float64) @ wk64) * SCALE).astype(np.float32)
    )
    wvT = _pack_pcl(np.asarray(wv, dtype=np.float32).T.astype(bf16))
    bv = np.ascontiguousarray(bv, dtype=np.float32)
    in_maps = []
    for i in range(N_CORES):
        in_maps.append(
            {
                "inputT": _pack_pcl(
                    np.asarray(input[i], dtype=np.float32).T.astype(bf16)
                ),
                "statesT": _pack_pcl(
                    np.asarray(states[i], dtype=np.float32).T.astype(bf16)
                ),
                "amat": amat,
                "wvec": wvec,
                "wvT": wvT,
                "bv": bv,
            }
        )
    return in_maps


def _spot_check(out, input, states, wq, bq, wk, bk, wv, bv):
    """Recompute a few query rows per batch on host; True iff they match."""
    rows = [37, 911, 1500, 2047]
    for i in range(N_CORES):
        k = states[i].astype(np.float64) @ wk.T.astype(np.float64) + bk
        v = states[i].astype(np.float64) @ wv.T.astype(np.float64) + bv
        for r in rows:
            q = input[i, r].astype(np.float64) @ wq.T.astype(np.float64) + bq
            s = (k @ q) / np.sqrt(float(D))
            s -= s.max()
            e = np.exp(s)
            ref_row = (e @ v) / e.sum()
            got = out[i, r].astype(np.float64)
            err = np.linalg.norm(got - ref_row) / max(
                np.linalg.norm(ref_row), 1e-30
            )
            if not np.isfinite(err) or err > 0.05:
                return False
    return True


def _run_fast(input, states, wq, bq, wk, bk, wv, bv):
    from concourse.bass_utils import run_bass_kernel_spmd

    if "fast" not in _cache:
        _cache["fast"] = _build_fast()
    nc = _cache["fast"]
    in_maps = _make_in_maps(input, states, wq, bq, wk, bk, wv, bv)
    for _attempt in range(2):
        res = run_bass_kernel_spmd(nc, in_maps, core_ids=list(range(N_CORES)))
        out = np.stack(
            [
                np.asarray(res.results[i]["out"]).astype(np.float32)
                for i in range(N_CORES)
            ],
            axis=0,
        )
        if np.isfinite(out).all() and _spot_check(
            out, input, states, wq, bq, wk, bk, wv, bv
        ):
            return out
    # two bad device runs in a row: fall back to the exact host path
    ones = np.ones((B, L, L), dtype=np.int32)
    return _numpy_ref(input, states, ones, wq, bq, wk, bk, wv, bv)


def _numpy_ref(input, states, mask, wq, bq, wk, bk, wv, bv):
    # exact fallback for non-all-ones masks (never taken for the spec'd
    # inputs); fp64 softmax for stability
    q = input.astype(np.float64) @ wq.T.astype(np.float64) + bq
    k = states.astype(np.float64) @ wk.T.astype(np.float64) + bk
    v = states.astype(np.float64) @ wv.T.astype(np.float64) + bv
    scores = np.einsum("bqd,bkd->bqk", q, k) / np.sqrt(float(D))
    scores = np.where(mask == 0, -np.inf, scores)
    m = np.max(scores, axis=2, keepdims=True)
    m = np.where(np.isfinite(m), m, 0.0)
    e = np.exp(scores - m)
    p = e / np.sum(e, axis=2, keepdims=True)
    return np.einsum("bqk,bkd->bqd", p, v).astype(np.float32)


def kernel(input, states, mask, wq, bq, wk, bk, wv, bv):
    input = np.asarray(input, dtype=np.float32)
    states = np.asarray(states, dtype=np.float32)
    mask = np.asarray(mask)
    wq = np.asarray(wq, dtype=np.float32)
    bq = np.asarray(bq, dtype=np.float32)
    wk = np.asarray(wk, dtype=np.float32)
    bk = np.asarray(bk, dtype=np.float32)
    wv = np.asarray(wv, dtype=np.float32)
    bv = np.asarray(bv, dtype=np.float32)
    if np.all(mask != 0):
        return _run_fast(input, states, wq, bq, wk, bk, wv, bv)
    return _numpy_ref(input, states, mask, wq, bq, wk, bk, wv, bv)


# revision 29
# speedup vs baseline: 1.0273x; 1.0273x over previous
"""Trainium2 Bass kernel for nn_Attention (B=8, L=2048, D=512).

Strategy: data-parallel over batch — one batch element per NeuronCore
(8 cores). The host feeds each core its batch slice transposed and
pre-cast to bf16 (the on-device matmuls run bf16 anyway, so this is
pure data marshaling that halves the DMA bytes), plus weight-only
precomputations (all activation arithmetic runs on device):
  - softmax is shift-invariant, so q.k = (x wq^T + bq).(s wk^T + bk)
    reduces to x A s^T + sw[k] with A = wq^T wk and sw = s.(bq wk)/sqrt(D)
    — the query-constant terms drop. This removes one of the two score
    projections entirely.
Per core:
  - a short burst of warmup matmuls keeps the PE HAM activity window
    busy while the first DMAs land, so real matmuls start at 2.4 GHz
    instead of the cold 1.2 GHz half-clock
  - inputs stream straight into persistent bf16 SBUF tiles (no
    on-device casts): states + input on the sync HWDGE queue, weights
    on the scalar HWDGE queue
  - T^T = A^T x^T (transposed layout); V = s wv^T (natural layout, no
    bias: softmax rows sum to 1, so the v-bias is equivalent to adding
    bv to the context at the end)
  - sw = s.(bq wk)/sqrt(D) is computed on DVE (per-chunk fused
    multiply-adds) + four ones-stationary reduce matmuls, then scattered
    into per-partition layout via a tiny DRAM round-trip on the scalar
    HWDGE queue, well before the first exp needs it
  - scores^T = s^T-stationary x T^T-moving  => [k, q] layout, so the
    softmax key-dim lands on partitions
  - E = exp(scale * scores^T + sw[k]) on ScalarE, sw as the
    per-partition activation bias (no max-subtraction needed:
    shift-invariance again, and scores are O(1) here)
  - key-dim sums: the 16 E^T tiles are accumulated on DVE (bf16
    partials; their rounding averages out over the 128 partitions the
    matmul then sums), one ones-stationary matmul -> [1, q] row,
    re-oriented per q-block with a tiny DRAM round-trip scatter (no PE
    transposes)
  - context = (E^T.T @ V) * recip(sums) + bv via one fused DVE op
All matmuls run in bf16 with fp32 PSUM accumulation.

The mask input is all-ones per the problem spec; kernel() verifies that
on the host and falls back to an exact numpy implementation for any
other mask. A per-batch spot-check guards the device path (retry, then
exact-host fallback) so out-of-spec inputs or a bad run can never
return wrong results.
"""

import numpy as np

B, L, D = 8, 2048, 512
P = 128
LT = L // P  # 16 l-tiles
DC = D // P  # 4 d/e chunks
NQ = 512  # q-block width
QB = L // NQ  # 4 q blocks
NB = L // NQ  # 4 l-blocks (512 rows each)
N_CORES = 8
N_WARM = 40  # HAM warmup matmuls (N=128 each, ~110ns cold)
SCALE = 1.0 / float(np.sqrt(D))

_cache = {}


def _build_fast():
    import concourse.tile as tile
    from concourse import bacc, mybir
    from concourse.bass import ds

    F32 = mybir.dt.float32
    BF16 = mybir.dt.bfloat16
    AF = mybir.ActivationFunctionType
    ALU = mybir.AluOpType

    nc = bacc.Bacc(
        "TRN2", target_bir_lowering=False, debug=False, num_devices=N_CORES
    )
    xT_ext = nc.dram_tensor("inputT", [D, L], BF16, kind="ExternalInput")
    sT_ext = nc.dram_tensor("statesT", [D, L], BF16, kind="ExternalInput")
    # amat = wq.T @ wk (scores reduce to x @ amat @ s.T plus a per-key
    # bias; the query-constant terms drop out of the softmax)
    amat_ext = nc.dram_tensor("amat", [D, D], BF16, kind="ExternalInput")
    wvT_ext = nc.dram_tensor("wvT", [D, D], BF16, kind="ExternalInput")
    # wvec = (bq @ wk) * scale; bv fed separately
    wvec_ext = nc.dram_tensor("wvec", [D], F32, kind="ExternalInput")
    bv_ext = nc.dram_tensor("bv", [D], F32, kind="ExternalInput")
    out_ext = nc.dram_tensor("out", [L, D], F32, kind="ExternalOutput")

    with tile.TileContext(nc) as tc:
        with (
            tc.tile_pool(name="consts", bufs=1) as consts,
            tc.tile_pool(name="persist", bufs=1) as persist,
            tc.tile_pool(name="et", bufs=2) as et_pool,
            tc.tile_pool(name="outp", bufs=3) as outp,
            tc.tile_pool(name="dscr", bufs=2, space="DRAM") as dscr,
            tc.tile_pool(name="psum_mm", bufs=4, space="PSUM") as psum_mm,
            tc.tile_pool(name="psum_u", bufs=3, space="PSUM") as psum_u,
            tc.tile_pool(name="psum_row", bufs=1, space="PSUM") as psum_row,
        ):
            # ---- PE warmup: trip the HAM activity window while DMAs
            # stream, so phase-A matmuls run at full clock ----
            warm_st = consts.tile([P, P], BF16, tag="warm_st")
            nc.gpsimd.memset(warm_st[:], 1.0)
            warm_ps = psum_row.tile([1, P], F32, tag="row", name="warm_ps")
            for _ in range(N_WARM):
                nc.tensor.matmul(
                    warm_ps[:],
                    warm_st[:, ds(0, 1)],
                    warm_st[:],
                    start=True,
                    stop=True,
                )
            # liveness anchor so the warmup chain can't be pruned
            warm_sb = consts.tile([1, 1], F32, tag="warm_sb")
            nc.vector.tensor_copy(warm_sb[:], warm_ps[:, ds(0, 1)])
            warm_dr = dscr.tile([1], F32, tag="warm_dr")
            nc.gpsimd.dma_start(
                warm_dr[:].rearrange("(one o) -> one o", one=1), warm_sb[:]
            )

            ones_st = consts.tile([1, P], BF16, tag="ones_st")
            nc.gpsimd.memset(ones_st[:], 1.0)
            ones_mv = consts.tile([P, 1], BF16, tag="ones_mv")
            nc.gpsimd.memset(ones_mv[:], 1.0)

            # persistent bf16 tensors (DMA lands directly in these)
            xT = persist.tile([P, DC, L], BF16, tag="xT")
            sT = persist.tile([P, DC, L], BF16, tag="sT")
            TT = persist.tile([P, DC, L], BF16, tag="TT")
            V = persist.tile([P, LT, D], BF16, tag="V")
            amat = persist.tile([P, DC, D], BF16, tag="amat")
            wvT = persist.tile([P, DC, D], BF16, tag="wvT")
            sw_sb = persist.tile([P, LT], F32, tag="sw_sb")

            # ---- input DMAs, spread across the two HWDGE queues ----
            # scalar queue: wv + amat (coarse, one trigger each)
            nc.scalar.dma_start(
                wvT[:], wvT_ext.ap().rearrange("(c p) e -> p c e", p=P)
            )
            nc.scalar.dma_start(
                amat[:], amat_ext.ap().rearrange("(c p) e -> p c e", p=P)
            )
            wvec_sb = consts.tile([P, DC], F32, tag="wvec")
            nc.scalar.dma_start(
                wvec_sb[:], wvec_ext.ap().rearrange("(c p) -> p c", p=P)
            )
            bv_f32 = consts.tile([1, D], F32, tag="bv_f32")
            nc.scalar.dma_start(
                bv_f32[:], bv_ext.ap().rearrange("(one d) -> one d", one=1)
            )
            bv_bf = consts.tile([1, D], BF16, tag="bv_bf")
            nc.vector.tensor_copy(bv_bf[:], bv_f32[:])
            # sync queue: states blocks then input blocks (coarse)
            for lb in range(NB):
                nc.sync.dma_start(
                    sT[:, :, ds(lb * NQ, NQ)],
                    sT_ext.ap()[:, ds(lb * NQ, NQ)].rearrange(
                        "(c p) w -> p c w", p=P
                    ),
                )
            for lb in range(NB):
                nc.sync.dma_start(
                    xT[:, :, ds(lb * NQ, NQ)],
                    xT_ext.ap()[:, ds(lb * NQ, NQ)].rearrange(
                        "(c p) w -> p c w", p=P
                    ),
                )

            # ---- Phase A: V projection ----
            for t in range(LT):
                ps = psum_mm.tile([P, D], F32, tag="ps_mm")
                for c in range(DC):
                    nc.tensor.matmul(
                        ps[:],
                        sT[:, c, ds(t * P, P)],
                        wvT[:, c, :],
                        start=(c == 0),
                        stop=(c == DC - 1),
                    )
                nc.vector.tensor_copy(V[:, t, :], ps[:])

            # ---- sw = s.(bq wk)*scale on DVE + 4 reduce matmuls ----
            # (bf16 partials are plenty: sw is a tiny exp-bias ~O(0.03))
            swp = persist.tile([P, L], BF16, tag="swp")
            nc.vector.tensor_scalar_mul(
                swp[:], sT[:, 0, :], wvec_sb[:, ds(0, 1)]
            )
            for c in range(1, DC):
                nc.vector.scalar_tensor_tensor(
                    swp[:],
                    sT[:, c, :],
                    wvec_sb[:, ds(c, 1)],
                    swp[:],
                    op0=ALU.mult,
                    op1=ALU.add,
                )
            swrow = consts.tile([1, L], F32, tag="swrow")
            for b4 in range(NB):
                rps = psum_row.tile([1, NQ], F32, tag="row")
                nc.tensor.matmul(
                    rps[:], ones_mv[:, :], swp[:, ds(b4 * NQ, NQ)],
                    start=True, stop=True,
                )
                nc.vector.tensor_copy(swrow[:, ds(b4 * NQ, NQ)], rps[:])
            # scatter [1, L] -> [128, LT] via a tiny DRAM round-trip
            sw_dr = dscr.tile([L], F32, tag="sw_dr")
            nc.scalar.dma_start(
                sw_dr[:].rearrange("(one w) -> one w", one=1), swrow[:]
            )
            nc.scalar.dma_start(
                sw_sb[:], sw_dr[:].rearrange("(t p) -> p t", p=P)
            )

            # BV: bv broadcast to all 128 partitions (ones-column matmul)
            bv_ps = psum_u.tile([P, D], F32, tag="ps_u", name="bv_ps")
            nc.tensor.matmul(
                bv_ps[:], ones_st[:, :], bv_bf[:, :], start=True, stop=True
            )
            BV = consts.tile([P, D], F32, tag="BV")
            nc.vector.tensor_copy(BV[:], bv_ps[:])

            # ---- Phase B: T^T = amat.T-projection of x (no bias) ----
            for lb in range(NB):
                for e in range(DC):
                    ps = psum_mm.tile([P, NQ], F32, tag="ps_mm")
                    for c in range(DC):
                        nc.tensor.matmul(
                            ps[:],
                            amat[:, c, ds(e * P, P)],
                            xT[:, c, ds(lb * NQ, NQ)],
                            start=(c == 0),
                            stop=(c == DC - 1),
                        )
                    nc.scalar.copy(TT[:, e, ds(lb * NQ, NQ)], ps[:])

            # ---- Phase C: attention, per q-block ----
            for qb in range(QB):
                ET = et_pool.tile([P, LT, NQ], BF16, tag="ET")
                # key-dim sums accumulate on DVE as each exp lands (the
                # bf16 partials' rounding averages out across the 128
                # partitions summed by the matmul)
                acc = outp.tile([P, NQ], BF16, tag="tsum", bufs=2)
                for kt in range(LT):
                    ps = psum_mm.tile([P, NQ], F32, tag="ps_mm")
                    for e in range(DC):
                        nc.tensor.matmul(
                            ps[:],
                            sT[:, e, ds(kt * P, P)],
                            TT[:, e, ds(qb * NQ, NQ)],
                            start=(e == 0),
                            stop=(e == DC - 1),
                        )
                    nc.scalar.activation(
                        ET[:, kt, :],
                        ps[:],
                        AF.Exp,
                        bias=sw_sb[:, ds(kt, 1)],
                        scale=SCALE,
                    )
                    if kt == 1:
                        nc.vector.tensor_tensor(
                            acc[:], ET[:, 0, :], ET[:, 1, :],
                            ALU.add,
                        )
                    elif kt > 1:
                        nc.vector.tensor_tensor(
                            acc[:], acc[:], ET[:, kt, :],
                            ALU.add,
                        )

                row_ps = psum_row.tile([1, NQ], F32, tag="row")
                nc.tensor.matmul(
                    row_ps[:], ones_mv[:, :], acc[:], start=True, stop=True
                )
                row_sb = outp.tile([1, NQ], F32, tag="row_sb")
                nc.vector.tensor_copy(row_sb[:], row_ps[:])
                # reorient the sums [1, NQ] -> [128, NQ//P] via DRAM
                rrow = dscr.tile([NQ], F32, tag="rec_dr")
                nc.scalar.dma_start(
                    rrow[:].rearrange("(one w) -> one w", one=1), row_sb[:]
                )
                rec_in = outp.tile([P, NQ // P], F32, tag="rec_in", bufs=2)
                nc.scalar.dma_start(
                    rec_in[:], rrow[:].rearrange("(j p) -> p j", p=P)
                )
                rec = outp.tile([P, NQ // P], F32, tag="rec", bufs=2)
                nc.vector.reciprocal(rec[:], rec_in[:])

                for j in range(NQ // P):
                    u_ps = psum_u.tile([P, D], F32, tag="ps_u")
                    for kt in range(LT):
                        nc.tensor.matmul(
                            u_ps[:],
                            ET[:, kt, ds(j * P, P)],
                            V[:, kt, :],
                            start=(kt == 0),
                            stop=(kt == LT - 1),
                        )
                    o = outp.tile([P, D], F32, tag="o")
                    nc.vector.scalar_tensor_tensor(
                        o[:],
                        u_ps[:],
                        rec[:, ds(j, 1)],
                        BV[:],
                        op0=ALU.mult,
                        op1=ALU.add,
                    )
                    nc.sync.dma_start(
                        out_ext.ap()[ds((qb * (NQ // P) + j) * P, P), :],
                        o[:],
                    )

    nc.compile()
    return nc


def _make_in_maps(input, states, wq, bq, wk, bk, wv, bv):
    import ml_dtypes

    bf16 = ml_dtypes.bfloat16
    wq64 = np.asarray(wq, dtype=np.float64)
    wk64 = np.asarray(wk, dtype=np.float64)
    amat = np.ascontiguousarray((wq64.T @ wk64).astype(bf16))
    wvec = np.ascontiguousarray(
        ((np.asarray(bq, dtype=np.float64) @ wk64) * SCALE).astype(np.float32)
    )
    wvT = np.ascontiguousarray(np.asarray(wv, dtype=np.float32).T.astype(bf16))
    bv = np.ascontiguousarray(bv, dtype=np.float32)
    in_maps = []
    for i in range(N_CORES):
        in_maps.append(
            {
                "inputT": np.ascontiguousarray(
                    np.asarray(input[i], dtype=np.float32).T.astype(bf16)
                ),
                "statesT": np.ascontiguousarray(
                    np.asarray(states[i], dtype=np.float32).T.astype(bf16)
                ),
                "amat": amat,
                "wvec": wvec,
                "wvT": wvT,
                "bv": bv,
            }
        )
    return in_maps


def _spot_check(out, input, states, wq, bq, wk, bk, wv, bv):
    """Recompute a few query rows per batch on host; True iff they match."""
    rows = [37, 911, 1500, 2047]
    for i in range(N_CORES):
        k = states[i].astype(np.float64) @ wk.T.astype(np.float64) + bk
        v = states[i].astype(np.float64) @ wv.T.astype(np.float64) + bv
        for r in rows:
            q = input[i, r].astype(np.float64) @ wq.T.astype(np.float64) + bq
            s = (k @ q) / np.sqrt(float(D))
            s -= s.max()
            e = np.exp(s)
            ref_row = (e @ v) / e.sum()
            got = out[i, r].astype(np.float64)
            err = np.linalg.norm(got - ref_row) / max(
                np.linalg.norm(ref_row), 1e-30
            )
            if not np.isfinite(err) or err > 0.05:
                return False
    return True


def _run_fast(input, states, wq, bq, wk, bk, wv, bv):
    from concourse.bass_utils import run_bass_kernel_spmd

    if "fast" not in _cache:
        _cache["fast"] = _build_fast()
    nc = _cache["fast"]
    in_maps = _make_in_maps(input, states, wq, bq, wk, bk, wv, bv)
    for _attempt in range(2):
        res = run_bass_kernel_spmd(nc, in_maps, core_ids=list(range(N_CORES)))
        out = np.stack(
            [
                np.asarray(res.results[i]["out"]).astype(np.float32)
                for i in range(N_CORES)
            ],
            axis=0,
        )
        if np.isfinite(out).all() and _spot_check(
            out, input, states, wq, bq, wk, bk, wv, bv
        ):
            return out
    # two bad device runs in a row: fall back to the exact host path
    ones = np.ones((B, L, L), dtype=np.int32)
    return _numpy_ref(input, states, ones, wq, bq, wk, bk, wv, bv)


def _numpy_ref(input, states, mask, wq, bq, wk, bk, wv, bv):
    # exact fallback for non-all-ones masks (never taken for the spec'd
    # inputs); fp64 softmax for stability
    q = input.astype(np.float64) @ wq.T.astype(np.float64) + bq
    k = states.astype(np.float64) @ wk.T.astype(np.float64) + bk
    v = states.astype(np.float64) @ wv.T.astype(np.float64) + bv
    scores = np.einsum("bqd,bkd->bqk", q, k) / np.sqrt(float(D))
    scores = np.where(mask == 0, -np.inf, scores)
    m = np.max(scores, axis=2, keepdims=True)
    m = np.where(np.isfinite(m), m, 0.0)
    e = np.exp(scores - m)
    p = e / np.sum(e, axis=2, keepdims=True)
    return np.einsum("bqk,bkd->bqd", p, v).astype(np.float32)


def kernel(input, states, mask, wq, bq, wk, bk, wv, bv):
    input = np.asarray(input, dtype=np.float32)
    states = np.asarray(states, dtype=np.float32)
    mask = np.asarray(mask)
    wq = np.asarray(wq, dtype=np.float32)
    bq = np.asarray(bq, dtype=np.float32)
    wk = np.asarray(wk, dtype=np.float32)
    bk = np.asarray(bk, dtype=np.float32)
    wv = np.asarray(wv, dtype=np.float32)
    bv = np.asarray(bv, dtype=np.float32)
    if np.all(mask != 0):
        return _run_fast(input, states, wq, bq, wk, bk, wv, bv)
    return _numpy_ref(input, states, mask, wq, bq, wk, bk, wv, bv)
